# revision 4
# baseline (speedup 1.0000x reference)
"""Grouped GEMM (MoE routing) Trainium2 kernel.

Expert-parallel across 8 NeuronCores with size-sorted slot assignment:
experts are sorted by token count and slot s on every core holds the
experts of size-rank [8s, 8s+8), so one SPMD program with per-slot
capacities cap_s = roundup4(max count in rank group) serves all cores.

Weights are streamed as fp8 E3M4 (scaled by 64, the 1/64 folded into
the bf16 x pack) to halve the dominant HBM traffic; the PE runs
bf16 (stationary x tile) x fp8e3 (moving w slab) matmuls at 1
col/cycle, accumulating over 20 K-chunks in PSUM ([128, 416] f32
tiles, 4 DOUT chunks), then evacuates to bf16 output (~1.2e-2 rel
err from the e3m4 weight quantization).

Both inputs are packed on the host in the exact SBUF layout
(partition-major), so every DMA line is contiguous (6.7-10KB per
partition) and descriptor counts stay minimal.
"""
import ml_dtypes
import numpy as np

import concourse.bass as bass
import concourse.mybir as mybir
import concourse.tile as tile
from concourse import bacc
from concourse.bass_utils import run_bass_kernel_spmd

G, T, DIN, DOUT = 64, 8192, 2560, 1664
NCORES = 8
EPC = G // NCORES   # expert slots per core
KC = DIN // 128     # 20 contraction chunks
NT = 4              # DOUT chunks
NW = DOUT // NT     # 416 (<=512 fp32 PSUM bank)
WSCALE = 64.0       # fp8 e3m4 weight scale (folded back via x/WSCALE)
KG = 4              # k-chunks per weight DMA

_cache = {}


def _build(caps):
    offs = np.concatenate([[0], np.cumsum(caps)]).astype(int)
    sumcap = int(offs[-1])
    nc = bacc.Bacc(trn_type="TRN2", debug=False)
    bf16 = mybir.dt.bfloat16
    f8 = mybir.dt.float8e3
    # xt: partition-major pack [128, KC*sumcap]; slot s occupies cols
    # [KC*offs[s], KC*offs[s+1]) as (k-chunk, token) row-major
    xt = nc.dram_tensor("xt", [128, KC * sumcap], bf16, kind="ExternalInput").ap()
    # w: partition-major pack per slot [EPC, 128, KC*DOUT]
    w = nc.dram_tensor("w", [EPC, 128, KC * DOUT], f8, kind="ExternalInput").ap()
    out = nc.dram_tensor(
        "out", [sumcap, DOUT], bf16, kind="ExternalOutput"
    ).ap()
    with tile.TileContext(nc) as tc:
        with (
            tc.tile_pool(name="xtp", bufs=1) as xt_pool,
            tc.tile_pool(name="wp", bufs=6) as w_pool,
            tc.tile_pool(name="op", bufs=4) as o_pool,
            tc.tile_pool(name="ps", bufs=1, space="PSUM") as ps_pool,
        ):
            xts = {}
            for s in range(EPC):
                cap = int(caps[s])
                off = int(offs[s])
                xts[s] = xt_pool.tile([128, KC * cap], bf16, tag=f"xt{s}",
                                      name=f"xt{s}")
                nc.gpsimd.dma_start(
                    xts[s][:], xt[:, KC * off:KC * (off + cap)]
                )
            for s in range(EPC):
                cap = int(caps[s])
                off = int(offs[s])
                mts = (cap + 127) // 128  # m-tiles in this slot
                xt_sb = xts[s]
                psums = {}
                for m in range(mts):
                    for n in range(NT):
                        psums[m, n] = ps_pool.tile(
                            [128, NW], mybir.dt.float32, tag=f"ps{m}{n}",
                            name=f"psum_{s}_{m}_{n}",
                        )
                for kg in range(KC // KG):
                    w_sb = w_pool.tile([128, KG * DOUT], f8, tag="w",
                                       name=f"w{s}_{kg}")
                    nc.sync.dma_start(
                        w_sb[:], w[s, :, kg * KG * DOUT:(kg + 1) * KG * DOUT]
                    )
                    for kk in range(KG):
                        k = kg * KG + kk
                        for m in range(mts):
                            msz = min(128, cap - m * 128)
                            for n in range(NT):
                                nc.tensor.matmul(
                                    psums[m, n][:msz],
                                    xt_sb[:, k * cap + m * 128:
                                          k * cap + m * 128 + msz],
                                    w_sb[:, kk * DOUT + n * NW:
                                         kk * DOUT + (n + 1) * NW],
                                    start=(k == 0),
                                    stop=(k == KC - 1),
                                )
                for m in range(mts):
                    msz = min(128, cap - m * 128)
                    o_sb = o_pool.tile([128, DOUT], bf16, tag="o",
                                       name=f"o_{s}_{m}")
                    for n in range(NT):
                        nc.vector.tensor_copy(
                            o_sb[:msz, n * NW:(n + 1) * NW], psums[m, n][:msz]
                        )
                    nc.scalar.dma_start(
                        out[off + m * 128: off + m * 128 + msz, :], o_sb[:msz]
                    )
    nc.compile()
    return nc


def _run(inputs, trace=False):
    x = np.asarray(inputs["input"], dtype=np.float32)
    w = np.ascontiguousarray(np.asarray(inputs["weight"], dtype=np.float32))
    counts = np.asarray(inputs["tokens_per_expert"], dtype=np.int64)
    starts = np.concatenate([[0], np.cumsum(counts)[:-1]])

    order = np.argsort(-counts, kind="stable")  # experts by size rank
    # slot s, core c -> expert order[s*NCORES + c]; capacity = rank-group max
    caps = tuple(
        int(np.ceil(max(1, counts[order[s * NCORES:(s + 1) * NCORES]].max()) / 4) * 4)
        for s in range(EPC)
    )
    offs = np.concatenate([[0], np.cumsum(caps)]).astype(int)
    sumcap = int(offs[-1])

    if caps not in _cache:
        _cache[caps] = _build(caps)
    nc = _cache[caps]

    xs = (x * (1.0 / WSCALE)).astype(ml_dtypes.bfloat16)
    # [T, DIN] -> per-expert transposed [DIN, cnt] -> [128, KC, cnt]
    w8all = (w * WSCALE).astype(ml_dtypes.float8_e3m4)
    # pre-tile weights: [DIN, DOUT] -> [128, KC*DOUT] partition-major
    w8t = np.ascontiguousarray(
        w8all.reshape(G, KC, 128, DOUT).transpose(0, 2, 1, 3)
    ).reshape(G, 128, KC * DOUT)

    in_maps = []
    for c in range(NCORES):
        xt_pack = np.zeros((128, KC * sumcap), dtype=ml_dtypes.bfloat16)
        w_pack = np.empty((EPC, 128, KC * DOUT), dtype=ml_dtypes.float8_e3m4)
        for s in range(EPC):
            g = int(order[s * NCORES + c])
            cnt = int(counts[g])
            cap = caps[s]
            if cnt:
                # x slice [cnt, DIN] -> [KC, 128, cnt] -> [128, KC, cnt]
                blk = np.zeros((128, KC, cap), dtype=ml_dtypes.bfloat16)
                blk[:, :, :cnt] = (
                    xs[starts[g]:starts[g] + cnt].T
                    .reshape(KC, 128, cnt).transpose(1, 0, 2)
                )
                xt_pack[:, KC * offs[s]:KC * offs[s + 1]] = \
                    blk.reshape(128, KC * cap)
            w_pack[s] = w8t[g]
        in_maps.append({"xt": xt_pack, "w": w_pack})

    kw = {"trace_cores": list(range(NCORES))} if trace else {}
    res = run_bass_kernel_spmd(nc, in_maps, core_ids=list(range(NCORES)),
                               trace=trace, **kw)

    out = np.empty((T, DOUT), dtype=np.float32)
    for c in range(NCORES):
        for s in range(EPC):
            g = int(order[s * NCORES + c])
            cnt = int(counts[g])
            if cnt:
                out[starts[g]:starts[g] + cnt] = \
                    res.results[c]["out"][offs[s]:offs[s] + cnt].astype(np.float32)
    return out, res


def kernel(**inputs) -> np.ndarray:
    return _run(inputs)[0]


# revision 5
# speedup vs baseline: 1.1566x; 1.1566x over previous
"""Grouped GEMM (MoE routing) Trainium2 kernel.

Expert-parallel across 8 NeuronCores with size-sorted slot assignment:
experts are sorted by token count and slot s on every core holds the
experts of size-rank [8s, 8s+8), so one SPMD program with per-slot
capacities cap_s = roundup4(max count in rank group) serves all cores.

Weights are streamed as fp8 E3M4 (scaled by 64, the 1/64 folded into
the bf16 x pack) to halve the dominant HBM traffic; the PE runs
bf16 (stationary x tile) x fp8e3 (moving w slab) matmuls at 1
col/cycle, accumulating over 20 K-chunks in PSUM ([128, 416] f32
tiles, 4 DOUT chunks), then evacuates to bf16 output (~1.2e-2 rel
err from the e3m4 weight quantization).

Both inputs are packed on the host in the exact SBUF layout
(partition-major), so every DMA line is contiguous (6.7-10KB per
partition) and descriptor counts stay minimal.
"""
import ml_dtypes
import numpy as np

import concourse.bass as bass
import concourse.mybir as mybir
import concourse.tile as tile
from concourse import bacc
from concourse.bass_utils import run_bass_kernel_spmd

G, T, DIN, DOUT = 64, 8192, 2560, 1664
NCORES = 8
EPC = G // NCORES   # expert slots per core
KC = DIN // 128     # 20 contraction chunks
NT = 4              # DOUT chunks
NW = DOUT // NT     # 416 (<=512 fp32 PSUM bank)
WSCALE = 64.0       # fp8 e3m4 weight scale (folded back via x/WSCALE)
KG = 1              # k-chunks per weight DMA

_cache = {}


def _build(caps):
    offs = np.concatenate([[0], np.cumsum(caps)]).astype(int)
    sumcap = int(offs[-1])
    nc = bacc.Bacc(trn_type="TRN2", debug=False)
    bf16 = mybir.dt.bfloat16
    f8 = mybir.dt.float8e3
    # xt: partition-major pack [128, KC*sumcap]; slot s occupies cols
    # [KC*offs[s], KC*offs[s+1]) as (k-chunk, token) row-major
    xt = nc.dram_tensor("xt", [128, KC * sumcap], bf16, kind="ExternalInput").ap()
    # w: partition-major pack per slot [EPC, 128, KC*DOUT]
    w = nc.dram_tensor("w", [EPC, 128, KC * DOUT], f8, kind="ExternalInput").ap()
    out = nc.dram_tensor(
        "out", [sumcap, DOUT], bf16, kind="ExternalOutput"
    ).ap()
    with tile.TileContext(nc) as tc:
        with (
            tc.tile_pool(name="xtp", bufs=1) as xt_pool,
            tc.tile_pool(name="wp", bufs=12) as w_pool,
            tc.tile_pool(name="op", bufs=4) as o_pool,
            tc.tile_pool(name="ps", bufs=1, space="PSUM") as ps_pool,
        ):
            xts = {}
            for s in range(EPC):
                cap = int(caps[s])
                off = int(offs[s])
                xts[s] = xt_pool.tile([128, KC * cap], bf16, tag=f"xt{s}",
                                      name=f"xt{s}")
                nc.gpsimd.dma_start(
                    xts[s][:], xt[:, KC * off:KC * (off + cap)]
                )
            for s in range(EPC):
                cap = int(caps[s])
                off = int(offs[s])
                mts = (cap + 127) // 128  # m-tiles in this slot
                xt_sb = xts[s]
                psums = {}
                for m in range(mts):
                    for n in range(NT):
                        psums[m, n] = ps_pool.tile(
                            [128, NW], mybir.dt.float32, tag=f"ps{m}{n}",
                            name=f"psum_{s}_{m}_{n}",
                        )
                for kg in range(KC // KG):
                    w_sb = w_pool.tile([128, KG * DOUT], f8, tag="w",
                                       name=f"w{s}_{kg}")
                    nc.sync.dma_start(
                        w_sb[:], w[s, :, kg * KG * DOUT:(kg + 1) * KG * DOUT]
                    )
                    for kk in range(KG):
                        k = kg * KG + kk
                        for m in range(mts):
                            msz = min(128, cap - m * 128)
                            for n in range(NT):
                                nc.tensor.matmul(
                                    psums[m, n][:msz],
                                    xt_sb[:, k * cap + m * 128:
                                          k * cap + m * 128 + msz],
                                    w_sb[:, kk * DOUT + n * NW:
                                         kk * DOUT + (n + 1) * NW],
                                    start=(k == 0),
                                    stop=(k == KC - 1),
                                )
                for m in range(mts):
                    msz = min(128, cap - m * 128)
                    o_sb = o_pool.tile([128, DOUT], bf16, tag="o",
                                       name=f"o_{s}_{m}")
                    for n in range(NT):
                        nc.vector.tensor_copy(
                            o_sb[:msz, n * NW:(n + 1) * NW], psums[m, n][:msz]
                        )
                    nc.scalar.dma_start(
                        out[off + m * 128: off + m * 128 + msz, :], o_sb[:msz]
                    )
    nc.compile()
    return nc


def _run(inputs, trace=False):
    x = np.asarray(inputs["input"], dtype=np.float32)
    w = np.ascontiguousarray(np.asarray(inputs["weight"], dtype=np.float32))
    counts = np.asarray(inputs["tokens_per_expert"], dtype=np.int64)
    starts = np.concatenate([[0], np.cumsum(counts)[:-1]])

    order = np.argsort(-counts, kind="stable")  # experts by size rank
    # slot s, core c -> expert order[s*NCORES + c]; capacity = rank-group max
    caps = tuple(
        int(np.ceil(max(1, counts[order[s * NCORES:(s + 1) * NCORES]].max()) / 4) * 4)
        for s in range(EPC)
    )
    offs = np.concatenate([[0], np.cumsum(caps)]).astype(int)
    sumcap = int(offs[-1])

    if caps not in _cache:
        _cache[caps] = _build(caps)
    nc = _cache[caps]

    xs = (x * (1.0 / WSCALE)).astype(ml_dtypes.bfloat16)
    # [T, DIN] -> per-expert transposed [DIN, cnt] -> [128, KC, cnt]
    w8all = (w * WSCALE).astype(ml_dtypes.float8_e3m4)
    # pre-tile weights: [DIN, DOUT] -> [128, KC*DOUT] partition-major
    w8t = np.ascontiguousarray(
        w8all.reshape(G, KC, 128, DOUT).transpose(0, 2, 1, 3)
    ).reshape(G, 128, KC * DOUT)

    in_maps = []
    for c in range(NCORES):
        xt_pack = np.zeros((128, KC * sumcap), dtype=ml_dtypes.bfloat16)
        w_pack = np.empty((EPC, 128, KC * DOUT), dtype=ml_dtypes.float8_e3m4)
        for s in range(EPC):
            g = int(order[s * NCORES + c])
            cnt = int(counts[g])
            cap = caps[s]
            if cnt:
                # x slice [cnt, DIN] -> [KC, 128, cnt] -> [128, KC, cnt]
                blk = np.zeros((128, KC, cap), dtype=ml_dtypes.bfloat16)
                blk[:, :, :cnt] = (
                    xs[starts[g]:starts[g] + cnt].T
                    .reshape(KC, 128, cnt).transpose(1, 0, 2)
                )
                xt_pack[:, KC * offs[s]:KC * offs[s + 1]] = \
                    blk.reshape(128, KC * cap)
            w_pack[s] = w8t[g]
        in_maps.append({"xt": xt_pack, "w": w_pack})

    kw = {"trace_cores": list(range(NCORES))} if trace else {}
    res = run_bass_kernel_spmd(nc, in_maps, core_ids=list(range(NCORES)),
                               trace=trace, **kw)

    out = np.empty((T, DOUT), dtype=np.float32)
    for c in range(NCORES):
        for s in range(EPC):
            g = int(order[s * NCORES + c])
            cnt = int(counts[g])
            if cnt:
                out[starts[g]:starts[g] + cnt] = \
                    res.results[c]["out"][offs[s]:offs[s] + cnt].astype(np.float32)
    return out, res


def kernel(**inputs) -> np.ndarray:
    return _run(inputs)[0]
